# revision 6
# baseline (speedup 1.0000x reference)
"""GCN actor kernel v2 for 8 TRN2 NeuronCores (Bass/Tile).

Math (mirrors the reference):
    deg[v]  = in-degree(v) + 1 (self loop);  dinv = deg^-1/2
    y[s]    = dinv[s] * x[s]                        (folded into table on host)
    acc[v]  = sum_{(s,v) in E} y[s] + y[v]          (segment sum + self loop)
    hps[v]  = acc[v] @ conv_w + (1/dinv[v]) * conv_b
    h[v]    = relu(dinv[v] * hps[v])
    pooled  = sum_v LN(h[v]) = ln_g * (sum_v h[v]*rstd[v] - sum_v mu[v]*rstd[v])
              + N * ln_b      (computed via two PE column matmuls)
    out     = tanh(relu(pooled @ w2 + b2) @ w3 + b3)

Segment sum: dst-sharded across cores.  Edge tokens are gathered per edge
(SWDGE dma_gather) and folded into a per-window PSUM accumulator by TensorE
matmuls.  Three token classes minimize DMA/Q7/DVE cost:

  * PAIR tokens: host computes a per-core permutation (chain) of the node
    table such that adjacent rows co-occur in dst windows; one 512-byte
    descriptor (elem_step=128, elem_size=256) fetches TWO edge messages at
    the cost-model price of one (the <512B DMA latency penalty is dodged).
    A pair tile is [128 descs, 256] = 256 tokens; folded with two one-hot
    S matmuls (S built on DVE by is_equal against iota).
  * IDENTITY singles: leftover tokens are laid out dst-aligned (round r of
    window j holds slot p's r-th leftover source), so S == identity: no DVE
    S-build; pad slots gather a zero row.
  * COMPACT singles: deep-tail leftovers, dst-sorted with DVE-built S
    (baseline scheme).

Self-loops arrive via one plain sequential DMA (no gather tokens).

SWDGE idx layout (HW-probed): idx i of an instruction lives at
[i % 16, i // 16] int16, rows 0-15 replicated into 16-31; 1024 idx / gather
instruction is a hard HW cap (2048 wedges the device).
"""

import numpy as np
import ml_dtypes

import concourse.bass as bass
import concourse.bacc as bacc
import concourse.tile as tile
import concourse.mybir as mybir
from concourse.bass_utils import run_bass_kernel_spmd
from concourse.masks import make_identity

F32 = mybir.dt.float32
BF16 = mybir.dt.bfloat16
I16 = mybir.dt.int16
NPBF = ml_dtypes.bfloat16

NCORES = 8
D = 128
DA = 64
LN_EPS = 1e-5
W = 128
CHT = 8            # tiles per gather chunk: 8*128 = 1024 idx cap
N_NODES = 50000
NPAD = 50176
NPC = NPAD // NCORES      # 6272
NTO = NPC // 128          # 49
HALF = NPAD // 2          # 25088 rows per table
TBL = HALF + 16           # + zero pad rows
ZROW = HALF               # a guaranteed-zero table row

# host cost weights (ns/token) used to choose identity-round cutoffs
_C_IDPAD = 2.9   # DMA+Q7 cost of a wasted identity pad token
_C_OVF = 0.32    # extra cost of a compact token vs an identity token
_PERM_KEY = "singles"  # none | total | singles | pairs
_SPILL_T = 88


# ------------------------------------------------------------------
# host: chain construction (per core)
# ------------------------------------------------------------------

def _build_chain(tok_src, tok_w, rng):
    """Greedy chain over sources: adjacent chain rows should share dst
    windows.  Returns pi, a permutation of 0..NPAD-1 (chain order)."""
    # per-(src, window) counts
    key = tok_src.astype(np.int64) * 64 + tok_w
    uk, cnt_of = np.unique(key, return_counts=True)
    us = (uk >> 6).astype(np.int64)
    uw = (uk & 63).astype(np.int64)
    # group by source
    first = np.r_[True, us[1:] != us[:-1]]
    srcs = us[first]
    starts = np.flatnonzero(first)
    ends = np.r_[starts[1:], len(us)]
    nsrc = len(srcs)

    mate = {}

    # ---- pass 1: weight-2 buckets (sources sharing a window PAIR) ----
    nw = ends - starts
    cand_b = []
    cand_s = []
    for k in range(2, int(nw.max()) + 1):
        sel = np.flatnonzero(nw == k)
        if not len(sel):
            continue
        idx = starts[sel][:, None] + np.arange(k)[None, :]
        wmat = uw[idx]                      # [nk, k]
        for a in range(k):
            for b in range(a + 1, k):
                cand_b.append(wmat[:, a] * 64 + wmat[:, b])
                cand_s.append(srcs[sel])
    if cand_b:
        cb = np.concatenate(cand_b)
        cs = np.concatenate(cand_s)
        order = np.argsort(cb, kind="stable")
        cb = cb[order]
        cs = cs[order]
        prev_b = -1
        pending = -1
        for i in range(len(cb)):
            s = cs[i]
            if s in mate:
                continue
            b = cb[i]
            if b != prev_b:
                prev_b = b
                pending = s
                continue
            if pending < 0 or pending == s or pending in mate:
                pending = s
                continue
            mate[pending] = s
            mate[s] = pending
            pending = -1

    # units: pairs + singletons; endpoints for merge pass
    units = []          # list of python lists (chain fragments)
    unit_of = {}
    for s in srcs:
        if s in mate:
            m = mate[s]
            if m > s:
                u = len(units)
                units.append([s, m])
                unit_of[s] = u
                unit_of[m] = u
        else:
            u = len(units)
            units.append([s])
            unit_of[s] = u

    # ---- pass 2: weight-1 endpoint merge (two rounds) ----
    parent = list(range(len(units)))

    def find(u):
        while parent[u] != u:
            parent[u] = parent[parent[u]]
            u = parent[u]
        return u

    # remaining per-source window counts after pair-internal consumption
    rem = {}
    for i in range(nsrc):
        s = srcs[i]
        rem[s] = dict(zip(uw[starts[i]:ends[i]].tolist(),
                          cnt_of[starts[i]:ends[i]].tolist()))
    for s in list(mate):
        m = mate[s]
        if m < s:
            continue
        ra, rb = rem[s], rem[m]
        for wv in list(ra):
            if wv in rb:
                t = min(ra[wv], rb[wv])
                ra[wv] -= t
                rb[wv] -= t

    for rnd in range(6):
        # endpoint -> window candidates; round 0 uses window PAIRS (weight 2)
        eb = []
        es = []
        for i in range(nsrc):
            s = srcs[i]
            u = find(unit_of[s])
            ch = units[u]
            if ch is None or (s != ch[0] and s != ch[-1]):
                continue
            ws = [wv for wv, cc in rem[s].items() if cc > 0]
            if rnd == 0:
                ws.sort()
                for a in range(len(ws)):
                    for bq in range(a + 1, len(ws)):
                        eb.append(int(ws[a]) * 64 + int(ws[bq]))
                        es.append(s)
            else:
                for wv in ws:
                    eb.append(wv)
                    es.append(s)
        if not eb:
            break
        eb = np.asarray(eb)
        es = np.asarray(es, dtype=np.int64)
        order = np.argsort(eb, kind="stable")
        eb = eb[order]
        es = es[order]
        prev_b = -1
        pend = -1
        merged = 0
        for i in range(len(eb)):
            s = es[i]
            b = eb[i]
            if b != prev_b:
                prev_b = b
                pend = s
                continue
            if pend < 0:
                pend = s
                continue
            ua = find(unit_of[pend])
            ub = find(unit_of[s])
            ca, cb_ = units[ua], units[ub]
            if (
                ua == ub or ca is None or cb_ is None
                or (pend != ca[0] and pend != ca[-1])
                or (s != cb_[0] and s != cb_[-1])
            ):
                pend = s
                continue
            # orient: ...pend][s...
            if pend == ca[0]:
                ca.reverse()
            if s != cb_[0]:
                cb_.reverse()
            ca.extend(cb_)
            units[ub] = None
            parent[ub] = ua
            ra, rb = rem[pend], rem[s]
            for wv in list(ra):
                if wv in rb:
                    t = min(ra[wv], rb[wv])
                    ra[wv] -= t
                    rb[wv] -= t
            merged += 1
            pend = -1

    chain = []
    for ch in units:
        if ch:
            chain.extend(ch)
    # inactive sources fill the remainder
    active = np.zeros(NPAD, bool)
    active[np.asarray(chain, dtype=np.int64)] = True
    rest = np.flatnonzero(~active)
    pi = np.concatenate([np.asarray(chain, dtype=np.int64), rest])
    assert len(pi) == NPAD
    return pi


def _assign_tokens(tok_src, tok_drel, pi):
    """Given chain pi, split tokens into pair descriptors and singles.

    Returns (pairs, singles):
      pairs:   list per half: (desc_pos int array, drel_lo, drel_hi) with
               window = drel_lo >> 7 (both tokens same window)
      singles: list per half: (pos, drel) arrays
    """
    pos_of = np.empty(NPAD, np.int64)
    pos_of[pi] = np.arange(NPAD)
    tok_pos = pos_of[tok_src]
    tok_w = tok_drel >> 7

    # group tokens by (pos, w); keep drel lists
    order = np.lexsort((tok_drel, tok_w, tok_pos))
    p_s = tok_pos[order]
    w_s = tok_w[order]
    d_s = tok_drel[order]

    # group boundaries for (pos, w)
    gkey = p_s * 64 + w_s
    gfirst = np.r_[True, gkey[1:] != gkey[:-1]]
    gstart = np.flatnonzero(gfirst)
    gcount = np.diff(np.r_[gstart, len(gkey)])
    gpos = p_s[gstart]
    gw = w_s[gstart]
    ngroups = len(gstart)

    # per-position group ranges
    pg_start = np.full(NPAD + 1, -1, np.int64)
    pfirst = np.r_[True, gpos[1:] != gpos[:-1]]
    pstarts = np.flatnonzero(pfirst)
    upos = gpos[pstarts]
    pg_lo = {int(upos[i]): (int(pstarts[i]),
                            int(pstarts[i + 1] if i + 1 < len(pstarts)
                                else ngroups))
             for i in range(len(upos))}

    used = np.zeros(ngroups, np.int64)   # consumed tokens per group

    pair_pos, pair_lo, pair_hi = [], [], []
    # L->R walk over junctions
    for k_half0 in range(2):
        lo = k_half0 * HALF
        hi = lo + HALF - 1
        for k in range(lo, hi):
            ga = pg_lo.get(k)
            gb = pg_lo.get(k + 1)
            if ga is None or gb is None:
                continue
            ia, ea = ga
            ib, eb = gb
            while ia < ea and ib < eb:
                wa, wb = gw[ia], gw[ib]
                if wa < wb:
                    ia += 1
                    continue
                if wb < wa:
                    ib += 1
                    continue
                ra = gcount[ia] - used[ia]
                rb = gcount[ib] - used[ib]
                m = min(ra, rb)
                if m > 0:
                    sa = gstart[ia] + used[ia]
                    sb = gstart[ib] + used[ib]
                    for t in range(m):
                        pair_pos.append(k)
                        pair_lo.append(d_s[sa + t])
                        pair_hi.append(d_s[sb + t])
                    used[ia] += m
                    used[ib] += m
                ia += 1
                ib += 1

    pair_pos = np.asarray(pair_pos, np.int64)
    pair_lo = np.asarray(pair_lo, np.int64)
    pair_hi = np.asarray(pair_hi, np.int64)

    # leftover singles
    rem_idx = []
    rem_pos = []
    rem_drel = []
    for g in range(ngroups):
        r = gcount[g] - used[g]
        if r > 0:
            s0 = gstart[g] + used[g]
            rem_idx.extend(range(s0, s0 + r))
    rem_idx = np.asarray(rem_idx, np.int64)
    s_pos = p_s[rem_idx] if len(rem_idx) else np.empty(0, np.int64)
    s_drel = d_s[rem_idx] if len(rem_idx) else np.empty(0, np.int64)

    pairs, singles = [], []
    for h in range(2):
        lo = h * HALF
        m = (pair_pos >= lo) & (pair_pos < lo + HALF)
        pairs.append((pair_pos[m] - lo, pair_lo[m], pair_hi[m]))
        ms = (s_pos >= lo) & (s_pos < lo + HALF)
        singles.append((s_pos[ms] - lo, s_drel[ms]))
    return pairs, singles


def _wrap16(a):
    """SWDGE idx layout: idx i at [i % 16, i // 16], rows replicated to
    16-31."""
    L = len(a)
    w = np.zeros((128, max(L // 16, 1)), np.int16)
    if L:
        w16 = np.ascontiguousarray(a.reshape(L // 16, 16).T)
        w[0:16] = w16
        w[16:32] = w16
    return w


def prep(x, edge_index, conv_w, conv_b, ln_g, ln_b, w2, b2, w3, b3):
    x = np.asarray(x, np.float32)
    ei = np.asarray(edge_index).astype(np.int64)
    conv_w = np.asarray(conv_w, np.float32)
    conv_b = np.asarray(conv_b, np.float32)
    ln_g = np.asarray(ln_g, np.float32)
    ln_b = np.asarray(ln_b, np.float32)
    w2 = np.asarray(w2, np.float32)
    b2 = np.asarray(b2, np.float32)
    w3 = np.asarray(w3, np.float32)
    b3 = np.asarray(b3, np.float32)

    N = x.shape[0]
    src, dst = ei[0], ei[1]
    deg = np.bincount(dst, minlength=NPAD).astype(np.float64) + 1.0
    deg[N:] = 1.0
    dinv = 1.0 / np.sqrt(deg)

    xs = np.zeros((NPAD, D), np.float32)
    xs[:N] = (x.astype(np.float64) * dinv[:N, None]).astype(np.float32)
    dinvf = dinv.astype(np.float32)
    dinvif = (1.0 / dinv).astype(np.float32)
    dinvf[N:] = 0.0
    dinvif[N:] = 0.0

    rng = np.random.default_rng(0)
    core = dst // NPC

    per_core = []
    invmaps = []
    for c in range(NCORES):
        m = core == c
        s = src[m]
        drel0 = dst[m] - c * NPC
        # degree-balanced node->(window, slot) relabeling: node ranked i by
        # in-degree goes to window i % NTO, slot i // NTO -> every window
        # gets an equal share of heavy/light nodes (uniform cell loads)
        degl = np.bincount(drel0, minlength=NPC)
        order = np.argsort(-degl, kind="stable")          # rank -> old rel
        nodemap = np.empty(NPC, np.int64)
        nodemap[order] = (np.arange(NPC) % NTO) * 128 + np.arange(NPC) // NTO
        inv = np.empty(NPC, np.int64)
        inv[nodemap] = np.arange(NPC)                     # new rel -> old rel
        drel = nodemap[drel0]
        pi = _build_chain(s, drel >> 7, rng)
        pairs, singles = _assign_tokens(s, drel, pi)
        invmaps.append(inv)
        per_core.append((pi, pairs, singles))

    # ---- pair-tile tail spill: a tail tile under half full costs more
    # DMA than gathering its tokens as singles, so spill those pairs ----
    npair = np.zeros((2, NTO), np.int64)
    for t in range(2):
        for j in range(NTO):
            mx = 0
            for c in range(NCORES):
                pw = per_core[c][1][t][1] >> 7
                mx = max(mx, int((pw == j).sum()))
            full, rem = divmod(mx, 128)
            npair[t, j] = full + (1 if rem >= _SPILL_T else 0)
    for c in range(NCORES):
        pi, prs, sgl = per_core[c]
        for t in range(2):
            ppos, plo, phi = prs[t]
            pw = plo >> 7
            keep = np.ones(len(ppos), bool)
            ex_pos, ex_dr = [], []
            for j in range(NTO):
                sel = np.flatnonzero(pw == j)
                cap = int(npair[t, j]) * 128
                if len(sel) > cap:
                    drop = sel[cap:]
                    keep[drop] = False
                    ex_pos.extend(ppos[drop])
                    ex_dr.extend(plo[drop])
                    ex_pos.extend(ppos[drop] + 1)
                    ex_dr.extend(phi[drop])
            prs[t] = (ppos[keep], plo[keep], phi[keep])
            if ex_pos:
                sp, sd = sgl[t]
                sgl[t] = (
                    np.concatenate([sp, np.asarray(ex_pos, np.int64)]),
                    np.concatenate([sd, np.asarray(ex_dr, np.int64)]),
                )
        per_core[c] = (pi, prs, sgl)

    # ---- identity-round cutoffs R[t][j], uniform across cores ----
    # per core/table/window/slot single counts
    slot_cnt = np.zeros((NCORES, 2, NTO, W), np.int64)
    for c in range(NCORES):
        _, _, sgl = per_core[c]
        for t in range(2):
            _, sd = sgl[t]
            if len(sd):
                np.add.at(slot_cnt[c, t], (sd >> 7, sd & 127), 1)

    R = np.zeros((2, NTO), np.int64)
    ncomp = np.zeros((2, NTO), np.int64)
    for t in range(2):
        for j in range(NTO):
            cnts = slot_cnt[:, t, j, :]           # [cores, 128]
            rmax = int(cnts.max())
            best, bestc = 0, None
            for r in range(rmax + 1):
                ovf_mx = int(np.maximum(cnts - r, 0).sum(axis=1).max())
                ntile = -(-ovf_mx // 128) if ovf_mx else 0
                # true quantized footprint: descs cost ~1.9ns each
                # (DMA+Q7), each compact tile adds a ~110ns DVE S-build
                cost = 1.9 * 128 * (r + ntile) + 110 * ntile
                if bestc is None or cost < bestc:
                    bestc, best = cost, r
            R[t, j] = best
            ovf_c = np.maximum(cnts - best, 0).sum(axis=1)   # per core
            ncomp[t, j] = -(-int(ovf_c.max()) // 128) if ovf_c.max() else 0

    cfg = {
        "R": tuple(map(tuple, R)),
        "ncomp": tuple(map(tuple, ncomp)),
        "npair": tuple(map(tuple, npair)),
    }

    # ---- emit per-core arrays ----
    iw = np.arange(W, dtype=np.float32).astype(NPBF)
    in_maps = []
    for c in range(NCORES):
        pi, prs, sgl = per_core[c]
        xa = np.zeros((TBL, D), NPBF)
        xa[:HALF] = xs[pi[:HALF]].astype(NPBF)
        xb = np.zeros((TBL, D), NPBF)
        xb[:HALF] = xs[pi[HALF:]].astype(NPBF)

        # self rows, arranged [128, NTO*128], relabeled node order
        inv = invmaps[c]
        xo = xs[c * NPC + inv].astype(NPBF)                  # [6272, 128]
        x_own = np.ascontiguousarray(
            xo.reshape(NTO, 128, D).transpose(1, 0, 2).reshape(128, NTO * D)
        )

        m = {}
        m["xa"] = xa
        m["xb"] = xb
        m["x_own"] = x_own
        m["iotaw"] = iw
        dloc = dinvf[c * NPC + inv]
        m["dinvo"] = np.ascontiguousarray(dloc.reshape(NTO, 128).T)
        m["dinvi"] = np.ascontiguousarray(
            dinvif[c * NPC + inv].astype(NPBF).reshape(1, NPC)
        )
        m["cbrow"] = conv_b.astype(NPBF).reshape(1, D)
        m["cwb"] = conv_w.astype(NPBF)
        m["gcol"] = ln_g
        m["lbs"] = (ln_b * float(N)).astype(np.float32)
        m["negones"] = np.full((1, D), -1.0, np.float32)
        m["w2"] = w2
        m["b2"] = b2
        m["w3"] = w3
        m["b3"] = b3

        for t, nm in ((0, "a"), (1, "b")):
            # ---- pair stream ----
            ppos, plo, phi = prs[t]
            pw = plo >> 7
            idx_list = []
            lo_list = []
            hi_list = []
            for j in range(NTO):
                sel = np.flatnonzero(pw == j)
                need = npair[t, j] * 128
                ip = np.full(need, ZROW, np.int64)
                il = np.full(need, -1.0, np.float32)
                ih = np.full(need, -1.0, np.float32)
                n = len(sel)
                ip[:n] = ppos[sel]
                il[:n] = (plo[sel] & 127).astype(np.float32)
                ih[:n] = (phi[sel] & 127).astype(np.float32)
                idx_list.append(ip)
                lo_list.append(il)
                hi_list.append(ih)
            allp = np.concatenate(idx_list) if idx_list else np.empty(0, np.int64)
            ntp = int(npair[t].sum())
            m[f"srcp{nm}"] = _wrap16(allp.astype(np.int16))
            dr = np.zeros((128, max(2 * ntp, 1)), np.float32)
            if ntp:
                lo_all = np.concatenate(lo_list).reshape(ntp, 128)
                hi_all = np.concatenate(hi_list).reshape(ntp, 128)
                dr[:, 0::2] = lo_all.T
                dr[:, 1::2] = hi_all.T
            m[f"drp{nm}"] = dr

            # ---- singles stream: identity rounds then compact, per j ----
            spos, sdrel = sgl[t]
            sw = sdrel >> 7
            sslot = sdrel & 127
            idx_s = []
            comp_idx = []
            comp_dr = []
            for j in range(NTO):
                sel = np.flatnonzero(sw == j)
                slot = sslot[sel]
                pos = spos[sel]
                o2 = np.lexsort((pos, slot))
                slot = slot[o2]
                pos = pos[o2]
                rank = np.arange(len(slot)) - np.searchsorted(slot, slot)
                rj = int(R[t, j])
                idm = rank < rj
                # identity tiles: idx[r*128 + p]
                idt = np.full((rj, 128), ZROW, np.int64)
                idt[rank[idm], slot[idm]] = pos[idm]
                idx_s.append(idt.reshape(-1))
                # compact overflow
                osel = ~idm
                cpos = pos[osel]
                cdr = (slot[osel]).astype(np.float32)
                need = ncomp[t, j] * 128
                cp = np.full(need, ZROW, np.int64)
                cd = np.full(need, -1.0, np.float32)
                cp[: len(cpos)] = cpos
                cd[: len(cpos)] = cdr
                comp_idx.append(cp)
                comp_dr.append(cd)
                # stream order per j: identity tiles then compact tiles
                idx_s.append(cp)
            # build interleaved stream: we appended id then compact per j
            alls = np.concatenate(idx_s) if idx_s else np.empty(0, np.int64)
            m[f"srcs{nm}"] = _wrap16(alls.astype(np.int16))
            ntc = int(ncomp[t].sum())
            dc = np.zeros((128, max(ntc, 1)), np.float32)
            if ntc:
                dca = np.concatenate(comp_dr).reshape(ntc, 128)
                dc[:] = dca.T
            m[f"drc{nm}"] = dc

        in_maps.append(m)
    return cfg, in_maps


# ------------------------------------------------------------------
# device graph
# ------------------------------------------------------------------

def build_graph(cfg):
    R = cfg["R"]
    ncomp = cfg["ncomp"]
    npair = cfg["npair"]
    ntp = [sum(npair[0]), sum(npair[1])]
    nts = [
        sum(R[0]) + sum(ncomp[0]),
        sum(R[1]) + sum(ncomp[1]),
    ]

    nc = bacc.Bacc(
        "TRN2",
        target_bir_lowering=False,
        debug=cfg.get("debug", False),
        num_devices=NCORES,
    )

    xa = nc.dram_tensor("xa", [TBL, D], BF16, kind="ExternalInput")
    xb = nc.dram_tensor("xb", [TBL, D], BF16, kind="ExternalInput")
    x_own = nc.dram_tensor("x_own", [128, NTO * D], BF16, kind="ExternalInput")
    iotaw = nc.dram_tensor("iotaw", [W], BF16, kind="ExternalInput")
    dinvo = nc.dram_tensor("dinvo", [128, NTO], F32, kind="ExternalInput")
    dinvi = nc.dram_tensor("dinvi", [1, NPC], BF16, kind="ExternalInput")
    cbrow = nc.dram_tensor("cbrow", [1, D], BF16, kind="ExternalInput")
    cwb = nc.dram_tensor("cwb", [D, D], BF16, kind="ExternalInput")
    gcol = nc.dram_tensor("gcol", [D], F32, kind="ExternalInput")
    lbs = nc.dram_tensor("lbs", [D], F32, kind="ExternalInput")
    negones = nc.dram_tensor("negones", [1, D], F32, kind="ExternalInput")
    w2 = nc.dram_tensor("w2", [D, D], F32, kind="ExternalInput")
    b2 = nc.dram_tensor("b2", [D], F32, kind="ExternalInput")
    w3 = nc.dram_tensor("w3", [D, DA], F32, kind="ExternalInput")
    b3 = nc.dram_tensor("b3", [DA], F32, kind="ExternalInput")

    srcp = [None, None]
    drp = [None, None]
    srcs_t = [None, None]
    drc = [None, None]
    for t, nm in ((0, "a"), (1, "b")):
        srcp[t] = nc.dram_tensor(
            f"srcp{nm}", [128, max(ntp[t] * 8, 1)], I16, kind="ExternalInput"
        )
        drp[t] = nc.dram_tensor(
            f"drp{nm}", [128, max(2 * ntp[t], 1)], F32, kind="ExternalInput"
        )
        srcs_t[t] = nc.dram_tensor(
            f"srcs{nm}", [128, max(nts[t] * 8, 1)], I16, kind="ExternalInput"
        )
        drc[t] = nc.dram_tensor(
            f"drc{nm}", [128, max(sum(ncomp[t]), 1)], F32, kind="ExternalInput"
        )

    out_ext = nc.dram_tensor("out", [DA, 1], F32, kind="ExternalOutput")
    if cfg.get("dump"):
        dbg_acc = nc.dram_tensor("dbg_acc", [128, NTO * D], F32,
                                 kind="ExternalOutput")
        dbg_h = nc.dram_tensor("dbg_h", [128, NTO * D], F32,
                               kind="ExternalOutput")
        dbg_pool = nc.dram_tensor("dbg_pool", [128, 2], F32,
                                  kind="ExternalOutput")
    cc_in = nc.dram_tensor("cc_in", [D, 1], F32)
    mus_dram = nc.dram_tensor("mus_dram", [1], F32)
    cc_out = nc.dram_tensor("cc_out", [D, 1], F32, addr_space="Shared")

    with tile.TileContext(nc) as tc:
        with tc.tile_pool(name="persist", bufs=1) as per:
            # idx tables first: unblock the gather pipeline ASAP
            IDXR = 128 if cfg.get("debug") else 32
            sp_t = [None, None]
            dp_t = [None, None]
            ss_t = [None, None]
            dc_t = [None, None]
            for t in range(2):
                sp_t[t] = per.tile(
                    [128, max(ntp[t] * 8, 1)], I16, name=f"sp{t}"
                )
                nc.sync.dma_start(
                    out=sp_t[t][:IDXR, :], in_=srcp[t][:IDXR, :]
                )
                ss_t[t] = per.tile(
                    [128, max(nts[t] * 8, 1)], I16, name=f"ss{t}"
                )
                nc.sync.dma_start(
                    out=ss_t[t][:IDXR, :], in_=srcs_t[t][:IDXR, :]
                )
            for t in range(2):
                dp_t[t] = per.tile(
                    [128, max(2 * ntp[t], 1)], F32, name=f"dp{t}"
                )
                nc.sync.dma_start(out=dp_t[t][:], in_=drp[t][:, :])
                dc_t[t] = per.tile(
                    [128, max(sum(ncomp[t]), 1)], F32, name=f"dc{t}"
                )
                nc.sync.dma_start(out=dc_t[t][:], in_=drc[t][:, :])
            identb = per.tile([128, 128], BF16)
            make_identity(nc, identb[:])
            iw_t = per.tile([128, W], BF16)
            nc.sync.dma_start(
                out=iw_t[:],
                in_=bass.AP(tensor=iotaw, offset=0, ap=[[0, 128], [1, W]]),
            )
            xo_t = per.tile([128, NTO * D], BF16)
            nc.sync.dma_start(out=xo_t[:], in_=x_own[:, :])
            cw_t = per.tile([D, D], BF16)
            nc.sync.dma_start(out=cw_t[:], in_=cwb[:, :])
            dinvo_t = per.tile([128, NTO], F32)
            nc.sync.dma_start(out=dinvo_t[:], in_=dinvo[:, :])
            dinvi_t = per.tile([1, NPC], BF16)
            nc.sync.dma_start(out=dinvi_t[:], in_=dinvi[:, :])
            cb_t = per.tile([1, D], BF16)
            nc.sync.dma_start(out=cb_t[:], in_=cbrow[:, :])
            no_t = per.tile([1, D], F32)
            nc.sync.dma_start(out=no_t[:], in_=negones[:, :])
            eps_t = per.tile([128, 1], F32)
            nc.vector.memset(eps_t[:], LN_EPS)
            # prefetch tail weights during the gather phase
            g_t = per.tile([128, 1], F32)
            nc.sync.dma_start(out=g_t[:], in_=gcol[:, None])
            lb_t = per.tile([128, 1], F32)
            nc.sync.dma_start(out=lb_t[:], in_=lbs[:, None])
            w2_t = per.tile([D, D], F32)
            nc.sync.dma_start(out=w2_t[:], in_=w2[:, :])
            b2_t = per.tile([D, 1], F32)
            nc.sync.dma_start(out=b2_t[:], in_=b2[:, None])
            w3_t = per.tile([D, DA], F32)
            nc.sync.dma_start(out=w3_t[:], in_=w3[:, :])
            b3_t = per.tile([DA, 1], F32)
            nc.sync.dma_start(out=b3_t[:], in_=b3[:, None])

            xt = [xa, xb]
            # per-(t, j) tile offsets in the pair / single streams
            pair_base = [np.cumsum([0] + list(npair[t])) for t in range(2)]
            sing_tiles = [
                [R[t][j] + ncomp[t][j] for j in range(NTO)] for t in range(2)
            ]
            sing_base = [np.cumsum([0] + sing_tiles[t]) for t in range(2)]
            comp_base = [np.cumsum([0] + list(ncomp[t])) for t in range(2)]

            with (
                tc.tile_pool(name="gp", bufs=10) as gpp,
                tc.tile_pool(name="gs", bufs=10) as gsp,
                tc.tile_pool(name="sb", bufs=24) as sbp,
                tc.tile_pool(name="bps", bufs=3, space="PSUM") as bps,
                tc.tile_pool(name="bpt", bufs=1, space="PSUM") as bpt,
                tc.tile_pool(name="bph", bufs=2, space="PSUM") as bph,
                tc.tile_pool(name="pool_ps", bufs=1, space="PSUM") as cpl,
                tc.tile_pool(name="cpl_sb", bufs=1) as cpl_sb,
                tc.tile_pool(name="chh", bufs=3) as chh,
                tc.tile_pool(name="cst", bufs=6) as cst,
            ):
                pool_sb = cpl_sb.tile([128, 1], F32, tag="pool_sb")
                nc.vector.memset(pool_sb[:], 0.0)
                mus_acc = cpl_sb.tile([1, 1], F32, tag="mus_acc")
                nc.vector.memset(mus_acc[:], 0.0)

                pchunks = [{}, {}]
                schunks = [{}, {}]

                def pair_chunk(t, c):
                    if c in pchunks[t]:
                        return pchunks[t][c]
                    n = min(CHT, ntp[t] - c * CHT)
                    g = gpp.tile([128, CHT, 2 * D], BF16, tag=f"gp{t}")
                    xp = bass.AP(
                        tensor=xt[t], offset=0, ap=[[D, TBL - 1], [1, 2 * D]]
                    )
                    nc.gpsimd.dma_gather(
                        g[:, :n, :],
                        xp,
                        sp_t[t][:, c * CHT * 8 : (c * CHT + n) * 8],
                        n * 128,
                        n * 128,
                        2 * D,
                        elem_step=D,
                    )
                    pchunks[t][c] = g
                    return g

                def sing_chunk(t, c):
                    if c in schunks[t]:
                        return schunks[t][c]
                    n = min(CHT, nts[t] - c * CHT)
                    g = gsp.tile([128, CHT, D], BF16, tag=f"gs{t}")
                    nc.gpsimd.dma_gather(
                        g[:, :n, :],
                        xt[t][:, :],
                        ss_t[t][:, c * CHT * 8 : (c * CHT + n) * 8],
                        n * 128,
                        n * 128,
                        D,
                    )
                    schunks[t][c] = g
                    return g

                for j in range(NTO):
                    ps = bps.tile([128, 128], F32, tag="ps")
                    # count matmuls to set stop on the last
                    nmm = 1
                    for t in range(2):
                        nmm += R[t][j] + 2 * npair[t][j] + ncomp[t][j]
                    mmi = 0

                    def flags():
                        nonlocal mmi
                        st = mmi == 0
                        sp = mmi == nmm - 1
                        mmi += 1
                        return st, sp

                    # identity rounds
                    for t in range(2):
                        for r in range(R[t][j]):
                            tile_id = int(sing_base[t][j]) + r
                            g = sing_chunk(t, tile_id // CHT)
                            k = tile_id % CHT
                            st, sp = flags()
                            nc.tensor.matmul(
                                ps[:],
                                lhsT=identb[:],
                                rhs=g[:, k, :],
                                start=st,
                                stop=sp,
                                skip_group_check=True,
                            )
                    # pair tiles
                    for t in range(2):
                        for u in range(npair[t][j]):
                            tile_id = int(pair_base[t][j]) + u
                            g = pair_chunk(t, tile_id // CHT)
                            k = tile_id % CHT
                            for half in range(2):
                                sm = sbp.tile([128, W], BF16, tag="sm")
                                nc.vector.tensor_scalar(
                                    out=sm[:],
                                    in0=iw_t[:],
                                    scalar1=dp_t[t][
                                        :, 2 * tile_id + half : 2 * tile_id + half + 1
                                    ],
                                    scalar2=None,
                                    op0=mybir.AluOpType.is_equal,
                                )
                                st, sp = flags()
                                nc.tensor.matmul(
                                    ps[:],
                                    lhsT=sm[:],
                                    rhs=g[:, k, half * D : (half + 1) * D],
                                    start=st,
                                    stop=sp,
                                    skip_group_check=True,
                                )
                    # compact tiles
                    for t in range(2):
                        for u in range(ncomp[t][j]):
                            tile_id = int(sing_base[t][j]) + R[t][j] + u
                            g = sing_chunk(t, tile_id // CHT)
                            k = tile_id % CHT
                            col = int(comp_base[t][j]) + u
                            sm = sbp.tile([128, W], BF16, tag="sm")
                            nc.vector.tensor_scalar(
                                out=sm[:],
                                in0=iw_t[:],
                                scalar1=dc_t[t][:, col : col + 1],
                                scalar2=None,
                                op0=mybir.AluOpType.is_equal,
                            )
                            st, sp = flags()
                            nc.tensor.matmul(
                                ps[:],
                                lhsT=sm[:],
                                rhs=g[:, k, :],
                                start=st,
                                stop=sp,
                                skip_group_check=True,
                            )
                    # self loop last (identity weights)
                    st, sp = flags()
                    nc.tensor.matmul(
                        ps[:],
                        lhsT=identb[:],
                        rhs=xo_t[:, j * D : (j + 1) * D],
                        start=st,
                        stop=sp,
                        skip_group_check=True,
                    )
                    assert mmi == nmm

                    # ---- transform + epilogue ----
                    axs = chh.tile([128, D], BF16, tag="axs")
                    nc.scalar.activation(
                        out=axs[:], in_=ps[:],
                        func=mybir.ActivationFunctionType.Copy,
                    )
                    if cfg.get("dump"):
                        dak = chh.tile([128, D], F32, tag="dak")
                        nc.vector.tensor_copy(out=dak[:], in_=ps[:])
                        nc.sync.dma_start(
                            out=dbg_acc[:, j * D : (j + 1) * D], in_=dak[:]
                        )
                    pst = bpt.tile([128, 128], BF16)
                    nc.tensor.transpose(out=pst[:], in_=axs[:], identity=identb[:])
                    axT = chh.tile([128, D], BF16, tag="axT")
                    nc.scalar.activation(
                        out=axT[:], in_=pst[:],
                        func=mybir.ActivationFunctionType.Copy,
                    )
                    hps = bph.tile([128, D], F32)
                    nc.tensor.matmul(
                        hps[:], lhsT=axT[:], rhs=cw_t[:], start=True, stop=False,
                        skip_group_check=True,
                    )
                    nc.tensor.matmul(
                        hps[:],
                        lhsT=dinvi_t[:, j * 128 : (j + 1) * 128],
                        rhs=cb_t[:],
                        start=False,
                        stop=True,
                        skip_group_check=True,
                    )
                    h = chh.tile([128, D], F32, tag="h")
                    nc.scalar.activation(
                        out=h[:], in_=hps[:],
                        func=mybir.ActivationFunctionType.Relu,
                        scale=dinvo_t[:, j : j + 1],
                    )
                    hb = chh.tile([128, D], BF16, tag="hb")
                    nc.scalar.activation(
                        out=hb[:], in_=hps[:],
                        func=mybir.ActivationFunctionType.Relu,
                        scale=dinvo_t[:, j : j + 1],
                    )
                    if cfg.get("dump"):
                        nc.sync.dma_start(
                            out=dbg_h[:, j * D : (j + 1) * D], in_=h[:]
                        )
                    stt = cst.tile([128, nc.vector.BN_STATS_DIM], F32)
                    nc.vector.bn_stats(out=stt[:], in_=h[:])
                    mv = cst.tile([128, nc.vector.BN_AGGR_DIM], F32)
                    nc.vector.bn_aggr(out=mv[:], in_=stt[:])
                    sq = cst.tile([128, 1], F32)
                    nc.scalar.activation(
                        out=sq[:],
                        in_=mv[:, 1:2],
                        func=mybir.ActivationFunctionType.Sqrt,
                        bias=eps_t[:],
                    )
                    rstd = cst.tile([128, 1], F32)
                    nc.vector.reciprocal(out=rstd[:], in_=sq[:])
                    rstdb = cst.tile([128, 1], BF16)
                    nc.vector.tensor_copy(out=rstdb[:], in_=rstd[:])
                    # pooled += h^T @ rstd ; musum += mu^T @ rstd
                    pj = cpl.tile([128, 1], F32, tag="pj")
                    nc.tensor.matmul(
                        pj[:], lhsT=hb[:], rhs=rstdb[:], start=True, stop=True,
                        skip_group_check=True,
                    )
                    nc.vector.tensor_add(
                        out=pool_sb[:], in0=pool_sb[:], in1=pj[:]
                    )
                    mj = cpl.tile([1, 1], F32, tag="mj")
                    nc.tensor.matmul(
                        mj[:], lhsT=mv[:, 0:1], rhs=rstd[:], start=True,
                        stop=True, skip_group_check=True,
                    )
                    nc.vector.tensor_add(
                        out=mus_acc[:], in0=mus_acc[:], in1=mj[:]
                    )

                # ---- tail ----
                mbc = cpl.tile([128, 1], F32, tag="pj")
                nc.tensor.matmul(
                    mbc[:], lhsT=no_t[:], rhs=mus_acc[:], start=True,
                    stop=True, skip_group_check=True,
                )
                zsum = per.tile([128, 1], F32)
                nc.vector.tensor_add(out=zsum[:], in0=pool_sb[:], in1=mbc[:])
                if cfg.get("dump"):
                    nc.sync.dma_start(out=dbg_pool[:, 0:1], in_=zsum[:])
                    nc.sync.dma_start(out=dbg_pool[0:1, 1:2], in_=mus_acc[:])

                nc.sync.dma_start(out=cc_in[:, :], in_=zsum[:])
                if cfg.get("single"):
                    nc.sync.dma_start(out=cc_out[:, :], in_=cc_in[:, :])
                else:
                    nc.gpsimd.collective_compute(
                        "AllReduce",
                        mybir.AluOpType.add,
                        replica_groups=[list(range(NCORES))],
                        ins=[cc_in.ap().opt()],
                        outs=[cc_out.ap().opt()],
                    )
                pooled = per.tile([128, 1], F32)
                nc.sync.dma_start(out=pooled[:], in_=cc_out[:, :])
                nc.vector.tensor_mul(out=pooled[:], in0=pooled[:], in1=g_t[:])
                nc.vector.tensor_add(out=pooled[:], in0=pooled[:], in1=lb_t[:])

                ps2 = cpl.tile([128, 1], F32, tag="pj")
                nc.tensor.matmul(
                    ps2[:], lhsT=w2_t[:], rhs=pooled[:], start=True, stop=True
                )
                a_t = per.tile([D, 1], F32)
                nc.scalar.activation(
                    out=a_t[:], in_=ps2[:],
                    func=mybir.ActivationFunctionType.Relu,
                    bias=b2_t[:],
                )
                ps3 = cpl.tile([128, 1], F32, tag="pj")
                nc.tensor.matmul(
                    ps3[:DA, :], lhsT=w3_t[:], rhs=a_t[:], start=True, stop=True
                )
                o_t = per.tile([DA, 1], F32)
                nc.scalar.activation(
                    out=o_t[:], in_=ps3[:DA, :],
                    func=mybir.ActivationFunctionType.Tanh,
                    bias=b3_t[:],
                )
                nc.sync.dma_start(out=out_ext[:, :], in_=o_t[:])

    nc.compile()
    return nc


_CACHE = {}


def kernel(**inputs):
    cfg, in_maps = prep(
        inputs["x"],
        inputs["edge_index"],
        inputs["conv_w"],
        inputs["conv_b"],
        inputs["ln_g"],
        inputs["ln_b"],
        inputs["w2"],
        inputs["b2"],
        inputs["w3"],
        inputs["b3"],
    )
    key = (cfg["R"], cfg["ncomp"], cfg["npair"])
    if key not in _CACHE:
        _CACHE[key] = build_graph(cfg)
    nc = _CACHE[key]
    res = run_bass_kernel_spmd(nc, in_maps, core_ids=list(range(NCORES)))
    return np.ascontiguousarray(
        res.results[0]["out"].astype(np.float32).reshape(1, DA)
    )
